# revision 29
# baseline (speedup 1.0000x reference)
# Masked multi-head attention for Trainium2, SPMD over 8 NeuronCores.
#
# Problem: q,k,v [2,16,2048,64] f32, mask [1,1,2048,2048] int32 (0/1),
#   out[b,h] = softmax(q@k^T/8 masked) @ v.
#
# Sharding: B*H = 32 heads, 4 per core (embarrassingly parallel).
#
# Per-head on-chip algorithm (no max-subtraction needed: scores ~ N(0,1),
# exp never overflows fp32; masked softmax == exp(S)*mask / sum(exp(S)*mask)):
#   Work in the transposed orientation S^T[k,q] so the softmax reduction
#   (over k) lands on the PE contraction dim instead of needing a
#   partition-axis reduction:
#     S^T[kc] (psum)  = kT[:,kc].T @ qT            (kc = 16 chunks of 128 k)
#     P^T[kc] (sbuf)  = exp(S^T[kc])·mask[kc]  (two paths, see below)
#     acc[65,1024]   += vp[kc].T @ P^T[kc]     (vp = [V | ones], fp32 psum)
#   acc rows 0..63 = (P@V)^T, row 64 = l = sum_k P.  Host divides and
#   transposes back.  The 1/sqrt(64) scale is folded into qT on the host.
#
# v2 structure (per head, per q-half "pass" of 1024):
#   - acc is [65, 1024] (2 psum banks), st pool is 3x[128,1024] (6 banks).
#   - QK chunk pair (2c, 2c+1) issues back-to-back on PE row-tiles
#     (rows 0-63 / 64-127, via base_partition auto tile_position) so the
#     two K=64 matmuls run concurrently in the array (~2x QK).
#   - P^T two paths per chunk:
#       ACT path:   pt = ScalarE.exp(st);  pt *= mask   (DVE or GpSimd)
#       SCHRAUD path (DVE only, one op):   pt_i16 = i16(st*1477.32 + B)
#         where B[k,q] = 15312 (fp16 exp bias, Schraudolph-corrected) when
#         mask=1 and -65504 when mask=0; i16 saturates to -32768 = fp16 -0
#         on masked entries (verified on HW), so bitcasting pt_i16 to fp16
#         yields exp(s)*mask with ~3% element error — well inside the 2e-2
#         budget after softmax averaging. This offloads exp from the
#         bottleneck ScalarE and needs NO separate mask multiply.
import os
from contextlib import ExitStack

import numpy as np
import ml_dtypes

B, H, S, D = 2, 16, 2048, 64
N_CORES = 8
HPC = (B * H) // N_CORES  # heads per core = 4
P = 128
NCHUNK = S // P  # 16
SPAN = 1024  # q-half width (one pass)
NPASS = S // SPAN  # 2

# Knobs.
QK_PACK = True  # row-tile QK chunk pairs (2x QK throughput)
SCHRAUD_CHUNKS = (1, 5, 9, 13)  # chunks on the DVE fused masked-exp path
POOL_MASK_CHUNKS = ()  # ACT-path chunks whose mask-mul goes to GpSimd
PT_BUFS = 10  # pt pool slots
ST_BUFS = 3  # score psum tiles (x2 banks)
IO_BUFS = 3  # qT/kT/vp buffer sets (head h+1 prefetches during head h)
MASK_BUFS = 2 * NCHUNK  # double-buffer the mask across For_i iterations
PV_PAIR_DELAY = 2  # emit PV of pair p after QK of pair p+delay
COPY_ENGINE = "act"  # "dve" | "act" — engine for the psum->sbuf out drain
SCHRAUD_SCALE = 1477.3197  # log2(e) * 2^10
SCHRAUD_B1 = 15312.0  # fp16 exponent-bias word minus Schraudolph correction
SCHRAUD_B0 = -65504.0  # drives i16 convert into -32768 saturation -> fp16 -0
# ablation knobs (bench-only attribution experiments; break correctness)
MASK_OFF = False
PV_OFF = False
QK_OFF = False

_CACHE = {}
LAST_RESULT = None  # BassKernelResults of the most recent run (for test.py)


def _build_nc(loop_reps=None):
    """Build the Bass program.  loop_reps=None -> the real kernel;
    loop_reps=K wraps the whole body in a hardware For_i loop (bench-only:
    lets wall-clock diffs between two K values measure per-iteration HW
    time through the slow axon tunnel)."""
    import concourse.bass as bass
    import concourse.tile as tile
    from concourse import bacc, mybir
    from concourse.alu_op_type import AluOpType

    F16 = mybir.dt.float16
    F32 = mybir.dt.float32
    I16 = mybir.dt.int16

    nc = bacc.Bacc("TRN2", target_bir_lowering=False, debug=False)

    qk_rows = 128 if QK_PACK else 64
    qT = nc.dram_tensor("qT", [HPC, qk_rows, S], F16, kind="ExternalInput").ap()
    kT = nc.dram_tensor("kT", [HPC, qk_rows, S], F16, kind="ExternalInput").ap()
    vp = nc.dram_tensor("vp", [HPC, S, D + 1], F16, kind="ExternalInput").ap()
    maskT = nc.dram_tensor("maskT", [S, S], F16, kind="ExternalInput").ap()
    o = nc.dram_tensor("o", [HPC, D + 1, S], F32, kind="ExternalOutput").ap()

    with tile.TileContext(nc) as tc, ExitStack() as ctx:
        mask_pool = ctx.enter_context(tc.tile_pool(name="mask", bufs=MASK_BUFS))
        io_pool = ctx.enter_context(tc.tile_pool(name="io", bufs=IO_BUFS))
        pt_pool = ctx.enter_context(tc.tile_pool(name="pt", bufs=PT_BUFS))
        out_pool = ctx.enter_context(tc.tile_pool(name="outsb", bufs=2))
        qk_psum = ctx.enter_context(tc.tile_pool(name="qk_psum", bufs=ST_BUFS, space="PSUM"))
        acc_psum = ctx.enter_context(tc.tile_pool(name="acc_psum", bufs=1, space="PSUM"))

        def load_io(h):
            qT_sb = io_pool.tile([qk_rows, S], F16, tag="qT", name=f"qT_sb{h}")
            nc.sync.dma_start(qT_sb[:], qT[h])
            kT_sb = io_pool.tile([qk_rows, S], F16, tag="kT", name=f"kT_sb{h}")
            nc.sync.dma_start(kT_sb[:], kT[h])
            vp_sb = io_pool.tile([P, NCHUNK, D + 1], F16, tag="vp", name=f"vp_sb{h}")
            nc.sync.dma_start(vp_sb[:], vp[h].rearrange("(c p) d -> p c d", p=P))
            return qT_sb, kT_sb, vp_sb

        def body(_iv=None):
            # head 0 IO first (QK(0) is the critical path at body start),
            # then the mask (consumed slightly later), then the other heads.
            io = {0: load_io(0)}
            mt = maskT.rearrange("(c p) q -> p c q", p=P)
            maskT_sb = []
            for c in range(NCHUNK):
                mtile = mask_pool.tile([P, S], F16, tag="mchunk", name=f"mask_c{c}")
                nc.sync.dma_start(mtile[:], mt[:, c, :])
                maskT_sb.append(mtile)
            for h in range(1, HPC):
                io[h] = load_io(h)

            # `defer` holds the previous pass's tail work: trailing PVs, the
            # psum->sbuf out copy, and the out DMA. It is flushed right after
            # the next pass's first QK pair so the PE/DVE/ACT never idle at a
            # pass boundary (PE: next QKs precede trailing PVs; DVE: the copy
            # sits behind the next pass's first mask ops, by which time the
            # PVs it waits on are done).
            defer = [None]
            copy_defer = [None]

            def flush_defer():
                # trailing PVs of the previous pass (right after the next
                # pass's first QK pair)
                if defer[0] is None:
                    return
                pend, acc, vp_tile, h, q_lo = defer[0]
                for c, ptap in pend:
                    emit_pv(acc, vp_tile, c, ptap)
                copy_defer[0] = (acc, h, q_lo)
                defer[0] = None

            def flush_copy():
                # the previous pass's psum->sbuf out drain + store
                if copy_defer[0] is None:
                    return
                acc, h, q_lo = copy_defer[0]
                out_sb = out_pool.tile([D + 1, SPAN], F32, tag="out", name=f"out{h}_{q_lo}")
                if PV_OFF:
                    nc.gpsimd.memset(out_sb[:], 0.0)
                elif COPY_ENGINE == "act":
                    nc.scalar.copy(out_sb[:], acc[:])
                else:
                    nc.vector.tensor_copy(out_sb[:], acc[:])
                nc.sync.dma_start(o[h][:, q_lo : q_lo + SPAN], out_sb[:])
                copy_defer[0] = None

            def emit_pv(acc, vp_tile, c, ptap):
                if PV_OFF:
                    return
                for qs in range(SPAN // 512):
                    nc.tensor.matmul(
                        acc[:, qs * 512 : (qs + 1) * 512],
                        lhsT=vp_tile[:, c, :],
                        rhs=ptap[:, qs * 512 : (qs + 1) * 512],
                        start=(c == 0),
                        stop=(c == NCHUNK - 1),
                    )

            for h in range(HPC):
                qT_sb, kT_sb, vp_sb = io[h]
                for ps in range(NPASS):
                    q_lo = ps * SPAN

                    def emit_qk_pair(c0, c1):
                        sts = []
                        for c in (c0, c1):
                            st = qk_psum.tile(
                                [P, SPAN], F32, tag="st", name=f"st{h}_{ps}_{c}")
                            sts.append(st)
                            if QK_OFF:
                                continue
                            r0 = 64 * (c % 2) if QK_PACK else 0
                            for j in range(SPAN // 512):
                                q0 = q_lo + j * 512
                                nc.tensor.matmul(
                                    st[:, j * 512 : (j + 1) * 512],
                                    lhsT=kT_sb[r0 : r0 + 64, c * P : (c + 1) * P],
                                    rhs=qT_sb[r0 : r0 + 64, q0 : q0 + 512],
                                    start=True,
                                    stop=True,
                                )
                        return sts

                    def emit_p(c, st):
                        msk = maskT_sb[c][:, q_lo : q_lo + SPAN]
                        if c in SCHRAUD_CHUNKS:
                            pt = pt_pool.tile([P, SPAN], I16, tag="pt", name=f"pts{h}_{ps}_{c}")
                            nc.vector.scalar_tensor_tensor(
                                pt[:], st[:], SCHRAUD_SCALE, msk,
                                op0=AluOpType.mult, op1=AluOpType.add,
                            )
                            return pt[:].bitcast(F16)
                        pt = pt_pool.tile([P, SPAN], F16, tag="pt", name=f"pt{h}_{ps}_{c}")
                        nc.scalar.activation(pt[:], st[:], mybir.ActivationFunctionType.Exp)
                        if not MASK_OFF:
                            eng = nc.gpsimd if c in POOL_MASK_CHUNKS else nc.vector
                            eng.tensor_mul(pt[:], pt[:], msk)
                        return pt[:]

                    acc = None
                    if not PV_OFF:
                        acc = acc_psum.tile([D + 1, SPAN], F32, tag="acc", name=f"acc{h}_{ps}")

                    pending = []  # [(c, pt_ap)] awaiting PV emission
                    for pr in range(NCHUNK // 2):
                        c0, c1 = 2 * pr, 2 * pr + 1
                        st0, st1 = emit_qk_pair(c0, c1)
                        if pr == 0:
                            # previous pass's trailing PVs, then its out
                            # drain+DMA (emitting the copy any later was
                            # measured slower on HW: 168.1us vs 143.9us)
                            flush_defer()
                            flush_copy()
                        pt0 = emit_p(c0, st0)
                        pt1 = emit_p(c1, st1)
                        pending.extend([(c0, pt0), (c1, pt1)])
                        while len(pending) > 2 * PV_PAIR_DELAY:
                            emit_pv(acc, vp_sb, *pending.pop(0))
                    defer[0] = (pending, acc, vp_sb, h, q_lo)
            flush_defer()
            flush_copy()

        if loop_reps is None:
            body()
        else:
            with tc.For_i(0, loop_reps, 1) as _i:
                body(_i)

    nc.compile()
    return nc


def _get_nc():
    if "nc" not in _CACHE:
        _CACHE["nc"] = _build_nc()
    return _CACHE["nc"]


def _prep_inputs(q, k, v, mask):
    """Host-side shard + layout prep. Returns one input map per core."""
    q = np.asarray(q, dtype=np.float32)
    k = np.asarray(k, dtype=np.float32)
    v = np.asarray(v, dtype=np.float32)
    mask = np.asarray(mask)

    # [B,H,S,D] -> [B*H, ...]
    qf = q.reshape(B * H, S, D)
    kf = k.reshape(B * H, S, D)
    vf = v.reshape(B * H, S, D)

    # transposed layouts; fold the 1/sqrt(D) scale into q before rounding
    qTf = np.ascontiguousarray(
        np.transpose(qf / np.sqrt(np.float32(D)), (0, 2, 1))
    ).astype(np.float16)  # [BH, 64, S]
    kTf = np.ascontiguousarray(np.transpose(kf, (0, 2, 1))).astype(np.float16)
    if QK_PACK:
        # duplicate rows so chunk pairs can use PE row-tiles (rows 0-63/64-127)
        qTf = np.concatenate([qTf, qTf], axis=1)  # [BH, 128, S]
        kTf = np.concatenate([kTf, kTf], axis=1)
    ones = np.ones((B * H, S, 1), np.float32)
    vpf = np.concatenate([vf, ones], axis=2).astype(np.float16)  # [BH, S, 65]

    # combined mask / Schraudolph-bias tensor, [S(k), S(q)] fp16:
    #   ACT chunks:     0.0 / 1.0   (multiplicand)
    #   SCHRAUD chunks: B0 / B1     (bias for the fused i16 masked-exp)
    m01 = mask[0, 0].T.astype(np.float32)  # [S(k), S(q)]
    maskT = m01.copy()
    for c in SCHRAUD_CHUNKS:
        rows = slice(c * P, (c + 1) * P)
        maskT[rows] = np.where(m01[rows] != 0, SCHRAUD_B1, SCHRAUD_B0)
    maskT = np.ascontiguousarray(maskT).astype(np.float16)

    in_maps = []
    for ci in range(N_CORES):
        sl = slice(ci * HPC, (ci + 1) * HPC)
        in_maps.append(
            {
                "qT": np.ascontiguousarray(qTf[sl]),
                "kT": np.ascontiguousarray(kTf[sl]),
                "vp": np.ascontiguousarray(vpf[sl]),
                "maskT": maskT,
            }
        )
    return in_maps


def kernel(q, k, v, mask):
    global LAST_RESULT
    from concourse import bass_utils

    nc = _get_nc()
    in_maps = _prep_inputs(q, k, v, mask)
    res = bass_utils.run_bass_kernel_spmd(
        nc, in_maps, core_ids=list(range(N_CORES))
    )
    LAST_RESULT = res

    out = np.empty((B * H, S, D), np.float32)
    for ci in range(N_CORES):
        oc = res.results[ci]["o"]  # [HPC, 65, S] f32
        num = oc[:, :D, :]  # (P@V)^T
        den = oc[:, D : D + 1, :]  # l
        out[ci * HPC : (ci + 1) * HPC] = np.transpose(num / den, (0, 2, 1))
    return out.reshape(B, H, S, D)


# revision 30
# speedup vs baseline: 1.2254x; 1.2254x over previous
# Masked multi-head attention for Trainium2, SPMD over 8 NeuronCores.
#
# Problem: q,k,v [2,16,2048,64] f32, mask [1,1,2048,2048] int32 (0/1),
#   out[b,h] = softmax(q@k^T/8 masked) @ v.
#
# Sharding: B*H = 32 heads, 4 per core (embarrassingly parallel).
#
# Per-head on-chip algorithm (no max-subtraction needed: scores ~ N(0,1),
# exp never overflows fp32; masked softmax == exp(S)*mask / sum(exp(S)*mask)):
#   Work in the transposed orientation S^T[k,q] so the softmax reduction
#   (over k) lands on the PE contraction dim instead of needing a
#   partition-axis reduction:
#     S^T[kc] (psum)  = kT[:,kc].T @ qT            (kc = 16 chunks of 128 k)
#     P^T[kc] (sbuf)  = exp(S^T[kc])·mask[kc]  (two paths, see below)
#     acc[65,1024]   += vp[kc].T @ P^T[kc]     (vp = [V | ones], fp32 psum)
#   acc rows 0..63 = (P@V)^T, row 64 = l = sum_k P.  Host divides and
#   transposes back.  The 1/sqrt(64) scale is folded into qT on the host.
#
# v2 structure (per head, per q-half "pass" of 1024):
#   - acc is [65, 1024] (2 psum banks), st pool is 3x[128,1024] (6 banks).
#   - QK chunk pair (2c, 2c+1) issues back-to-back on PE row-tiles
#     (rows 0-63 / 64-127, via base_partition auto tile_position) so the
#     two K=64 matmuls run concurrently in the array (~2x QK).
#   - P^T two paths per chunk:
#       ACT path:   pt = ScalarE.exp(st);  pt *= mask   (DVE or GpSimd)
#       SCHRAUD path (DVE only, one op):   pt_i16 = i16(st*1477.32 + B)
#         where B[k,q] = 15312 (fp16 exp bias, Schraudolph-corrected) when
#         mask=1 and -65504 when mask=0; i16 saturates to -32768 = fp16 -0
#         on masked entries (verified on HW), so bitcasting pt_i16 to fp16
#         yields exp(s)*mask with ~3% element error — well inside the 2e-2
#         budget after softmax averaging. This offloads exp from the
#         bottleneck ScalarE and needs NO separate mask multiply.
import os
from contextlib import ExitStack

import numpy as np
import ml_dtypes

B, H, S, D = 2, 16, 2048, 64
N_CORES = 8
HPC = (B * H) // N_CORES  # heads per core = 4
P = 128
NCHUNK = S // P  # 16
SPAN = 1024  # q-half width (one pass)
NPASS = S // SPAN  # 2

# Knobs.
QK_PACK = True  # row-tile QK chunk pairs (2x QK throughput)
SCHRAUD_CHUNKS = (1, 5, 9, 13)  # chunks on the DVE fused masked-exp path
POOL_MASK_CHUNKS = ()  # ACT-path chunks whose mask-mul goes to GpSimd
PT_BUFS = 10  # pt pool slots
ST_BUFS = 3  # score psum tiles (x2 banks)
IO_BUFS = 3  # qT/kT/vp buffer sets (head h+1 prefetches during head h)
MASK_BUFS = 2 * NCHUNK  # double-buffer the mask across For_i iterations
PV_PAIR_DELAY = 2  # emit PV of pair p after QK of pair p+delay
COPY_ENGINE = "dve"  # "dve" | "act" — engine for the psum->sbuf out drain
SCHRAUD_SCALE = 1477.3197  # log2(e) * 2^10
SCHRAUD_B1 = 15312.0  # fp16 exponent-bias word minus Schraudolph correction
SCHRAUD_B0 = -65504.0  # drives i16 convert into -32768 saturation -> fp16 -0
# ablation knobs (bench-only attribution experiments; break correctness)
MASK_OFF = False
PV_OFF = False
QK_OFF = False

_CACHE = {}
LAST_RESULT = None  # BassKernelResults of the most recent run (for test.py)


def _build_nc(loop_reps=None):
    """Build the Bass program.  loop_reps=None -> the real kernel;
    loop_reps=K wraps the whole body in a hardware For_i loop (bench-only:
    lets wall-clock diffs between two K values measure per-iteration HW
    time through the slow axon tunnel)."""
    import concourse.bass as bass
    import concourse.tile as tile
    from concourse import bacc, mybir
    from concourse.alu_op_type import AluOpType

    F16 = mybir.dt.float16
    F32 = mybir.dt.float32
    I16 = mybir.dt.int16

    nc = bacc.Bacc("TRN2", target_bir_lowering=False, debug=False)

    qk_rows = 128 if QK_PACK else 64
    qT = nc.dram_tensor("qT", [HPC, qk_rows, S], F16, kind="ExternalInput").ap()
    kT = nc.dram_tensor("kT", [HPC, qk_rows, S], F16, kind="ExternalInput").ap()
    vp = nc.dram_tensor("vp", [HPC, S, D + 1], F16, kind="ExternalInput").ap()
    maskT = nc.dram_tensor("maskT", [S, S], F16, kind="ExternalInput").ap()
    o = nc.dram_tensor("o", [HPC, D + 1, S], F32, kind="ExternalOutput").ap()

    with tile.TileContext(nc) as tc, ExitStack() as ctx:
        mask_pool = ctx.enter_context(tc.tile_pool(name="mask", bufs=MASK_BUFS))
        io_pool = ctx.enter_context(tc.tile_pool(name="io", bufs=IO_BUFS))
        pt_pool = ctx.enter_context(tc.tile_pool(name="pt", bufs=PT_BUFS))
        out_pool = ctx.enter_context(tc.tile_pool(name="outsb", bufs=2))
        qk_psum = ctx.enter_context(tc.tile_pool(name="qk_psum", bufs=ST_BUFS, space="PSUM"))
        acc_psum = ctx.enter_context(tc.tile_pool(name="acc_psum", bufs=1, space="PSUM"))

        def load_io(h):
            qT_sb = io_pool.tile([qk_rows, S], F16, tag="qT", name=f"qT_sb{h}")
            nc.sync.dma_start(qT_sb[:], qT[h])
            kT_sb = io_pool.tile([qk_rows, S], F16, tag="kT", name=f"kT_sb{h}")
            nc.sync.dma_start(kT_sb[:], kT[h])
            vp_sb = io_pool.tile([P, NCHUNK, D + 1], F16, tag="vp", name=f"vp_sb{h}")
            nc.sync.dma_start(vp_sb[:], vp[h].rearrange("(c p) d -> p c d", p=P))
            return qT_sb, kT_sb, vp_sb

        def body(_iv=None):
            # head 0 IO first (QK(0) is the critical path at body start),
            # then the mask (consumed slightly later), then the other heads.
            io = {0: load_io(0)}
            mt = maskT.rearrange("(c p) q -> p c q", p=P)
            maskT_sb = []
            for c in range(NCHUNK):
                mtile = mask_pool.tile([P, S], F16, tag="mchunk", name=f"mask_c{c}")
                nc.sync.dma_start(mtile[:], mt[:, c, :])
                maskT_sb.append(mtile)
            for h in range(1, HPC):
                io[h] = load_io(h)

            # `defer` holds the previous pass's tail work: trailing PVs, the
            # psum->sbuf out copy, and the out DMA. It is flushed right after
            # the next pass's first QK pair so the PE/DVE/ACT never idle at a
            # pass boundary (PE: next QKs precede trailing PVs; DVE: the copy
            # sits behind the next pass's first mask ops, by which time the
            # PVs it waits on are done).
            defer = [None]
            copy_defer = [None]

            def flush_defer():
                # trailing PVs of the previous pass (right after the next
                # pass's first QK pair)
                if defer[0] is None:
                    return
                pend, acc, vp_tile, h, q_lo = defer[0]
                for c, ptap in pend:
                    emit_pv(acc, vp_tile, c, ptap)
                copy_defer[0] = (acc, h, q_lo)
                defer[0] = None

            def flush_copy():
                # the previous pass's psum->sbuf out drain + store
                if copy_defer[0] is None:
                    return
                acc, h, q_lo = copy_defer[0]
                out_sb = out_pool.tile([D + 1, SPAN], F32, tag="out", name=f"out{h}_{q_lo}")
                if PV_OFF:
                    nc.gpsimd.memset(out_sb[:], 0.0)
                elif COPY_ENGINE == "act":
                    nc.scalar.copy(out_sb[:], acc[:])
                else:
                    nc.vector.tensor_copy(out_sb[:], acc[:])
                nc.sync.dma_start(o[h][:, q_lo : q_lo + SPAN], out_sb[:])
                copy_defer[0] = None

            def emit_pv(acc, vp_tile, c, ptap):
                if PV_OFF:
                    return
                for qs in range(SPAN // 512):
                    nc.tensor.matmul(
                        acc[:, qs * 512 : (qs + 1) * 512],
                        lhsT=vp_tile[:, c, :],
                        rhs=ptap[:, qs * 512 : (qs + 1) * 512],
                        start=(c == 0),
                        stop=(c == NCHUNK - 1),
                    )

            for h in range(HPC):
                qT_sb, kT_sb, vp_sb = io[h]
                for ps in range(NPASS):
                    q_lo = ps * SPAN

                    def emit_qk_pair(c0, c1):
                        sts = []
                        for c in (c0, c1):
                            st = qk_psum.tile(
                                [P, SPAN], F32, tag="st", name=f"st{h}_{ps}_{c}")
                            sts.append(st)
                            if QK_OFF:
                                continue
                            r0 = 64 * (c % 2) if QK_PACK else 0
                            for j in range(SPAN // 512):
                                q0 = q_lo + j * 512
                                nc.tensor.matmul(
                                    st[:, j * 512 : (j + 1) * 512],
                                    lhsT=kT_sb[r0 : r0 + 64, c * P : (c + 1) * P],
                                    rhs=qT_sb[r0 : r0 + 64, q0 : q0 + 512],
                                    start=True,
                                    stop=True,
                                )
                        return sts

                    def emit_p(c, st):
                        msk = maskT_sb[c][:, q_lo : q_lo + SPAN]
                        if c in SCHRAUD_CHUNKS:
                            pt = pt_pool.tile([P, SPAN], I16, tag="pt", name=f"pts{h}_{ps}_{c}")
                            nc.vector.scalar_tensor_tensor(
                                pt[:], st[:], SCHRAUD_SCALE, msk,
                                op0=AluOpType.mult, op1=AluOpType.add,
                            )
                            return pt[:].bitcast(F16)
                        pt = pt_pool.tile([P, SPAN], F16, tag="pt", name=f"pt{h}_{ps}_{c}")
                        nc.scalar.activation(pt[:], st[:], mybir.ActivationFunctionType.Exp)
                        if not MASK_OFF:
                            eng = nc.gpsimd if c in POOL_MASK_CHUNKS else nc.vector
                            eng.tensor_mul(pt[:], pt[:], msk)
                        return pt[:]

                    acc = None
                    if not PV_OFF:
                        acc = acc_psum.tile([D + 1, SPAN], F32, tag="acc", name=f"acc{h}_{ps}")

                    pending = []  # [(c, pt_ap)] awaiting PV emission
                    for pr in range(NCHUNK // 2):
                        c0, c1 = 2 * pr, 2 * pr + 1
                        st0, st1 = emit_qk_pair(c0, c1)
                        if pr == 0:
                            # previous pass's trailing PVs, then its out
                            # drain+DMA (emitting the copy any later was
                            # measured slower on HW: 168.1us vs 143.9us)
                            flush_defer()
                            flush_copy()
                        pt0 = emit_p(c0, st0)
                        pt1 = emit_p(c1, st1)
                        pending.extend([(c0, pt0), (c1, pt1)])
                        while len(pending) > 2 * PV_PAIR_DELAY:
                            emit_pv(acc, vp_sb, *pending.pop(0))
                    defer[0] = (pending, acc, vp_sb, h, q_lo)
            flush_defer()
            flush_copy()

        if loop_reps is None:
            body()
        else:
            with tc.For_i(0, loop_reps, 1) as _i:
                body(_i)

    nc.compile()
    return nc


def _get_nc():
    if "nc" not in _CACHE:
        _CACHE["nc"] = _build_nc()
    return _CACHE["nc"]


def _prep_inputs(q, k, v, mask):
    """Host-side shard + layout prep. Returns one input map per core."""
    q = np.asarray(q, dtype=np.float32)
    k = np.asarray(k, dtype=np.float32)
    v = np.asarray(v, dtype=np.float32)
    mask = np.asarray(mask)

    # [B,H,S,D] -> [B*H, ...]
    qf = q.reshape(B * H, S, D)
    kf = k.reshape(B * H, S, D)
    vf = v.reshape(B * H, S, D)

    # transposed layouts; fold the 1/sqrt(D) scale into q before rounding
    qTf = np.ascontiguousarray(
        np.transpose(qf / np.sqrt(np.float32(D)), (0, 2, 1))
    ).astype(np.float16)  # [BH, 64, S]
    kTf = np.ascontiguousarray(np.transpose(kf, (0, 2, 1))).astype(np.float16)
    if QK_PACK:
        # duplicate rows so chunk pairs can use PE row-tiles (rows 0-63/64-127)
        qTf = np.concatenate([qTf, qTf], axis=1)  # [BH, 128, S]
        kTf = np.concatenate([kTf, kTf], axis=1)
    ones = np.ones((B * H, S, 1), np.float32)
    vpf = np.concatenate([vf, ones], axis=2).astype(np.float16)  # [BH, S, 65]

    # combined mask / Schraudolph-bias tensor, [S(k), S(q)] fp16:
    #   ACT chunks:     0.0 / 1.0   (multiplicand)
    #   SCHRAUD chunks: B0 / B1     (bias for the fused i16 masked-exp)
    m01 = mask[0, 0].T.astype(np.float32)  # [S(k), S(q)]
    maskT = m01.copy()
    for c in SCHRAUD_CHUNKS:
        rows = slice(c * P, (c + 1) * P)
        maskT[rows] = np.where(m01[rows] != 0, SCHRAUD_B1, SCHRAUD_B0)
    maskT = np.ascontiguousarray(maskT).astype(np.float16)

    in_maps = []
    for ci in range(N_CORES):
        sl = slice(ci * HPC, (ci + 1) * HPC)
        in_maps.append(
            {
                "qT": np.ascontiguousarray(qTf[sl]),
                "kT": np.ascontiguousarray(kTf[sl]),
                "vp": np.ascontiguousarray(vpf[sl]),
                "maskT": maskT,
            }
        )
    return in_maps


def kernel(q, k, v, mask):
    global LAST_RESULT
    from concourse import bass_utils

    nc = _get_nc()
    in_maps = _prep_inputs(q, k, v, mask)
    res = bass_utils.run_bass_kernel_spmd(
        nc, in_maps, core_ids=list(range(N_CORES))
    )
    LAST_RESULT = res

    out = np.empty((B * H, S, D), np.float32)
    for ci in range(N_CORES):
        oc = res.results[ci]["o"]  # [HPC, 65, S] f32
        num = oc[:, :D, :]  # (P@V)^T
        den = oc[:, D : D + 1, :]  # l
        out[ci * HPC : (ci + 1) * HPC] = np.transpose(num / den, (0, 2, 1))
    return out.reshape(B, H, S, D)
